# revision 38
# baseline (speedup 1.0000x reference)
"""Multi-head attention layer on 8 TRN2 NeuronCores.

Reference computation (fp32):
    q = query @ Wq + bq; k = key @ Wk + bk; v = value @ Wv + bv
    scores = softmax(q @ k.T / sqrt(64)) per head
    out = (scores @ v) @ Wo + bo

Sharding: core c = 2*b + hh handles batch b and head-half hh (8 heads,
feature columns hh*512..(hh+1)*512). Each core computes its q/k/v
projections, attention for its 8 heads, and a partial output projection
against its 512-row slice of Wo; the host sums the two partials per batch.

Per-core design (cost-model-driven; TimelineSim 380.6us vs 436.6us for
the previous kernel):
  - bf16 everywhere (rel err 4.4e-3). qT/kT [128, 4, L] feature-major;
    head h lives at partitions (h%2)*64.. of feature tile h//2, so the
    K=64 score matmuls slice them directly.
  - exp on ScalarE in [128,1024] tiles (one per (head, lkt-pair,
    lq512-chunk)): 256 ACTs ~ 267us busy - the target engine to keep fed.
  - AV uses the o-layout: o[Lq, 64+1] accumulated per (head, lq128) as
    four 65-col strips in ONE PSUM bank. The first AV matmul opens the
    bank's psum accumulation group with start=True (hardware
    pending-zeroes the whole 2KB region, which is exactly what the other
    strips then accumulate onto); the last one closes it with stop=True.
    The ones column of v gives the softmax sums per partition, so
    normalization is one strided reciprocal [128,4] + 4 fused
    scale-copies (tensor_scalar mult) - no DRAM round trips.
  - oT for the output projection comes from PE transposes against a
    host-provided identity ([128,64] -> [64,128] bf16 into a rotating
    PSUM bank) + one DVE copy. (dma_start_transpose/XBAR produces wrong
    results on HW for <128-partition or non-contiguous destinations -
    it NaN'd here - so PE transpose it is.)
  - Head pairs alternate per kp step so each head's AV runs under the
    other head's exp; lq512-chunk outer/lkt inner keeps only 2
    accumulator banks live. k/v/q projections are emitted just-in-time
    along the attention frontier, extra work rides a filler queue pumped
    once per step.
PSUM: scores 2x[128,1024] (4 banks) + acc 2x[128,512] (2) +
proj/oproj/transpose 2x[128,512] (2) = 8 banks exactly.
"""

import numpy as np
import ml_dtypes

import concourse.bacc as bacc
import concourse.bass as bass
import concourse.mybir as mybir
import concourse.tile as tile
from concourse import bass_utils

B, L, DIM = 4, 2048, 1024
H, HD = 16, 64
N_CORES = 8
HL = 8             # local heads per core
FD = 512           # local feature columns
KT = DIM // 128    # 8 contraction k-tiles for projections
G = 4              # qF/kF free slots (head-dim sub-blocks)
MT = FD // 128     # 4 oT feature tiles
NLK = L // 128     # 16 Lk tiles
NC = L // 512      # 4 Lq column chunks
VSTR = 66          # per-head stride in v_sb (64 vals + ones col + pad)

BF16 = mybir.dt.bfloat16
FP8 = mybir.dt.float8e4
F32 = mybir.dt.float32
AF = mybir.ActivationFunctionType
DR = mybir.MatmulPerfMode.DoubleRow
ADD = mybir.AluOpType.add
SUB = mybir.AluOpType.subtract
MULT = mybir.AluOpType.mult


def _build_body(tc, io):
    nc = tc.nc
    (xq, xk, xv, wqf, wkf, wv, wo, bqf, bkf, bo, bvr, ident, outT) = io

    from contextlib import ExitStack
    with ExitStack() as ctx:
        const = ctx.enter_context(tc.tile_pool(name="const", bufs=1))
        wpool = ctx.enter_context(tc.tile_pool(name="wpool", bufs=1))
        xqpool = ctx.enter_context(tc.tile_pool(name="xqpool", bufs=2))
        xkpool = ctx.enter_context(tc.tile_pool(name="xkpool", bufs=4))
        xvpool = ctx.enter_context(tc.tile_pool(name="xvpool", bufs=2))
        qk_sb = ctx.enter_context(tc.tile_pool(name="qk_sb", bufs=1))
        e_pool = ctx.enter_context(tc.tile_pool(name="e_pool", bufs=6))
        osb_pool = ctx.enter_context(tc.tile_pool(name="osb", bufs=4))
        rec_pool = ctx.enter_context(tc.tile_pool(name="rec", bufs=4))
        stage = ctx.enter_context(tc.tile_pool(name="stage", bufs=3))
        spool = ctx.enter_context(
            tc.tile_pool(name="spool", bufs=2, space="PSUM"))
        apool = ctx.enter_context(
            tc.tile_pool(name="apool", bufs=2, space="PSUM"))
        ppool = ctx.enter_context(
            tc.tile_pool(name="ppool", bufs=2, space="PSUM"))

        # ---- constants ----
        bq_sb = const.tile([128, G], F32)
        nc.sync.dma_start(out=bq_sb, in_=bqf)
        bk_sb = const.tile([128, G], F32)
        nc.sync.dma_start(out=bk_sb, in_=bkf)
        bo_sb = const.tile([128, KT], F32)
        nc.sync.dma_start(out=bo_sb, in_=bo)
        bv_row = const.tile([1, FD], BF16)
        nc.sync.dma_start(out=bv_row, in_=bvr)
        ones_col = const.tile([1, 128], BF16)
        nc.vector.memset(ones_col, 1.0)
        ident_sb = const.tile([128, 128], BF16)
        nc.sync.dma_start(out=ident_sb, in_=ident)

        # ---- persistent activations ----
        qT = qk_sb.tile([128, G, L], BF16)
        kT = qk_sb.tile([128, G, L], BF16)
        v_sb = qk_sb.tile([128, NLK, HL * VSTR], BF16)
        oT_all = qk_sb.tile([128, MT, L], BF16)

        for h in range(HL):
            nc.vector.memset(v_sb[:, :, h * VSTR + 64:h * VSTR + 65], 1.0)

        # ---- weights: wq/wk [128, G, KT, 128] (per-g loads), wv/wo ----
        wq_sb = wpool.tile([128, G, KT, 128], BF16, tag="wq")
        wk_sb = wpool.tile([128, G, KT, 128], BF16, tag="wk")
        wv_sb = wpool.tile([128, KT, FD], BF16, tag="wv")
        wo_sb = wpool.tile([128, MT, DIM], BF16, tag="wo")
        wq_loaded = [False] * G
        wk_loaded = [False] * G
        wv_loaded = [False]
        wo_loaded = [False]

        def ensure_wq(g):
            if not wq_loaded[g]:
                nc.sync.dma_start(out=wq_sb[:, g], in_=wqf[:, g])
                wq_loaded[g] = True

        def ensure_wk(g):
            if not wk_loaded[g]:
                nc.sync.dma_start(out=wk_sb[:, g], in_=wkf[:, g])
                wk_loaded[g] = True

        def ensure_wv():
            if not wv_loaded[0]:
                nc.sync.dma_start(out=wv_sb, in_=wv)
                wv_loaded[0] = True

        def ensure_wo():
            if not wo_loaded[0]:
                nc.sync.dma_start(out=wo_sb, in_=wo)
                wo_loaded[0] = True

        # ---- x chunk loads (one DMA each: [128, KT, 512]) ----
        xq_ch, xk_ch, xv_ch = {}, {}, {}

        def load_chunk(pool, cache, dram, c, tag):
            if c not in cache:
                t = pool.tile([128, KT, 512], BF16, tag="x",
                              name=f"{tag}{c}")
                for kt in range(KT):
                    nc.sync.dma_start(
                        out=t[:, kt, :],
                        in_=dram[kt][:, c * 512:(c + 1) * 512])
                cache[c] = t
            return cache[c]

        # ---- projection units (one PSUM bank each) ----
        qk_done = set()   # ("q"|"k", c_or_d, g)
        v_done = set()    # lkt

        def proj_unit(kind, c, g):
            """q or k projection for feature tile g, column chunk c."""
            if (kind, c, g) in qk_done:
                return
            qk_done.add((kind, c, g))
            if kind == "q":
                ensure_wq(g)
                x_t = load_chunk(xqpool, xq_ch, xq, c, "xq")
                w_t, b_t, dst = wq_sb, bq_sb, qT
            else:
                ensure_wk(g)
                x_t = load_chunk(xkpool, xk_ch, xk, c, "xk")
                w_t, b_t, dst = wk_sb, bk_sb, kT
            ps = ppool.tile([128, 512], F32, tag="p", name=f"{kind}p{c}{g}")
            for kt in range(KT):
                nc.tensor.matmul(ps, w_t[:, g, kt, :], x_t[:, kt, :],
                                 start=(kt == 0), stop=(kt == KT - 1))
            nc.vector.tensor_scalar(
                out=dst[:, g, c * 512:(c + 1) * 512], in0=ps,
                scalar1=b_t[:, g:g + 1], scalar2=None, op0=ADD)

        def vproj_unit(lkt):
            if lkt in v_done:
                return
            v_done.add(lkt)
            ensure_wv()
            x_t = load_chunk(xvpool, xv_ch, xv, lkt // 4, "xv")
            t = lkt % 4
            ps = ppool.tile([128, 512], F32, tag="p", name=f"vp{lkt}")
            for kt in range(KT):
                nc.tensor.matmul(ps, x_t[:, kt, t * 128:(t + 1) * 128],
                                 wv_sb[:, kt, :],
                                 start=(kt == 0), stop=False)
            nc.tensor.matmul(ps, ones_col, bv_row, start=False, stop=True)
            dst = v_sb[:, lkt, :].rearrange(
                "p (h d) -> p h d", d=VSTR)[:, :, 0:64]
            nc.vector.tensor_copy(
                out=dst, in_=ps.rearrange("p (h d) -> p h d", d=64))

        # ---- attention pieces ----
        def scores_exp(h, c, kp):
            """Scores (one K=64 bf16 matmul per lkt) + one [128,1024] exp."""
            mt, p0 = h // 2, (h % 2) * 64
            s_ps = spool.tile([128, 1024], F32, tag="s", name=f"s{h}{c}{kp}")
            for j in (0, 1):
                lkt = 2 * kp + j
                nc.tensor.matmul(
                    s_ps[:, j * 512:(j + 1) * 512],
                    kT[p0:p0 + 64, mt, lkt * 128:(lkt + 1) * 128],
                    qT[p0:p0 + 64, mt, c * 512:(c + 1) * 512],
                    start=True, stop=True)
            e_t = e_pool.tile([128, 2, 512], BF16, tag="e", name=f"e{h}{kp}")
            nc.scalar.activation(e_t.rearrange("p a b -> p (a b)"), s_ps,
                                 AF.Exp, scale=0.125)
            return e_t

        def av(h, acc, e_t, kp):
            # the (kp0, j0, sub0) matmul opens the bank's psum group
            # (pending-zeroing the whole 2KB region, which is what the
            # other strips then accumulate onto); the last one closes it
            for j in (0, 1):
                lkt = 2 * kp + j
                va = v_sb[:, lkt, h * VSTR:h * VSTR + 65]
                for sub in range(4):
                    first = kp == 0 and j == 0 and sub == 0
                    last = kp == 7 and j == 1 and sub == 3
                    nc.tensor.matmul(
                        acc[:, sub * 128:sub * 128 + 65],
                        e_t[:, j, sub * 128:(sub + 1) * 128], va,
                        start=first, stop=last,
                        skip_group_check=not (first or last))

        def norm_transpose(h, c, acc):
            """1/sums, scale, and XBAR-transpose into oT_all."""
            rec4 = rec_pool.tile([128, G, 1], F32, tag="r", name=f"r{h}{c}")
            sums = acc.rearrange("p (s x) -> p s x", x=128)[:, :, 64:65]
            nc.vector.reciprocal(out=rec4, in_=sums)
            o_sb = osb_pool.tile([128, 4, 64], BF16, tag="o",
                                 name=f"o{h}{c}")
            for sub in range(4):
                nc.vector.tensor_scalar(
                    out=o_sb[:, sub, :],
                    in0=acc[:, sub * 128:sub * 128 + 64],
                    scalar1=rec4[:, sub, :], scalar2=None, op0=MULT)
            hp = (h % 2) * 64
            # PE transpose via identity: o_sb [128,64] -> [64,128] in PSUM
            tp = ppool.tile([128, 1024], BF16, tag="p", name=f"tp{h}{c}")
            for sub in range(4):
                nc.tensor.transpose(
                    tp[hp:hp + 64, sub * 128:(sub + 1) * 128],
                    o_sb[:, sub, :], ident_sb)
            nc.vector.tensor_copy(
                out=oT_all[hp:hp + 64, h // 2, c * 512:(c + 1) * 512],
                in_=tp[hp:hp + 64, 0:512])

        def oproj_unit(c, mt):
            ps = ppool.tile([128, 512], F32, tag="p", name=f"op{c}{mt}")
            for kt in range(MT):
                nc.tensor.matmul(ps, wo_sb[:, kt, mt * 128:(mt + 1) * 128],
                                 oT_all[:, kt, c * 512:(c + 1) * 512],
                                 start=(kt == 0), stop=(kt == MT - 1))
            st = stage.tile([128, 512], F32, tag="st", name=f"st{c}{mt}")
            nc.vector.tensor_scalar(
                out=st, in0=ps, scalar1=bo_sb[:, mt:mt + 1], scalar2=None,
                op0=ADD)
            nc.sync.dma_start(
                out=outT[mt * 128:(mt + 1) * 128, c * 512:(c + 1) * 512],
                in_=st)

        # ---- orchestration ----
        fillers = []

        def pump(n=1):
            for _ in range(min(n, len(fillers))):
                fillers.pop(0)()

        # c=0 warmup front: just enough for the first ACT + first AV
        proj_unit("q", 0, 0)
        proj_unit("k", 0, 0)
        vproj_unit(0)
        vproj_unit(1)

        for c in range(NC):
            if c == 0:
                # remaining q slots of chunk 0 (before any xq recycling),
                # then wo for the first oproj units
                for g in range(1, G):
                    fillers.append(lambda g=g: proj_unit("q", 0, g))
            else:
                # any stragglers (normally already pumped as fillers)
                for g in range(G):
                    proj_unit("q", c, g)
                for mt in range(KT):
                    fillers.append(lambda c=c, mt=mt: oproj_unit(c - 1, mt))
            if c + 1 < NC:
                for g in range(G):
                    fillers.append(
                        lambda c=c, g=g: proj_unit("q", c + 1, g))
            if c == 0:
                fillers.append(ensure_wo)

            for pair in range(HL // 2):
                hA, hB = 2 * pair, 2 * pair + 1
                accs = {}
                for h in (hA, hB):
                    # zeroing comes from the first AV matmul's start=True
                    # (pending-zeroes the whole bank region)
                    accs[h] = apool.tile([128, 512], F32, tag="a",
                                         name=f"acc{h}{c}")
                pend = []
                for kp in range(HL):
                    if c == 0:
                        # JIT: kproj for this pair's scores, v for the AVs
                        proj_unit("k", kp // 2, pair)
                        vproj_unit(2 * kp)
                        vproj_unit(2 * kp + 1)
                    for h in (hA, hB):
                        e_t = scores_exp(h, c, kp)
                        pend.append((h, e_t, kp))
                    # AV one step behind: both heads of the previous kp
                    while len(pend) > 2:
                        h, e_t, kpp = pend.pop(0)
                        av(h, accs[h], e_t, kpp)
                    pump(1)
                for h, e_t, kpp in pend:
                    av(h, accs[h], e_t, kpp)
                norm_transpose(hA, c, accs[hA])
                norm_transpose(hB, c, accs[hB])

        for mt in range(KT):
            oproj_unit(NC - 1, mt)
        while fillers:
            pump(1)


_CACHED = {}


def _get_nc():
    if "nc" not in _CACHED:
        nc = bacc.Bacc("TRN2", target_bir_lowering=False, debug=False)
        io = (
            nc.dram_tensor("xq", [KT, 128, L], BF16, kind="ExternalInput").ap(),
            nc.dram_tensor("xk", [KT, 128, L], BF16, kind="ExternalInput").ap(),
            nc.dram_tensor("xv", [KT, 128, L], BF16, kind="ExternalInput").ap(),
            nc.dram_tensor("wqf", [128, G, KT, 128], BF16,
                           kind="ExternalInput").ap(),
            nc.dram_tensor("wkf", [128, G, KT, 128], BF16,
                           kind="ExternalInput").ap(),
            nc.dram_tensor("wv", [128, KT, FD], BF16,
                           kind="ExternalInput").ap(),
            nc.dram_tensor("wo", [128, MT, DIM], BF16,
                           kind="ExternalInput").ap(),
            nc.dram_tensor("bqf", [128, G], F32, kind="ExternalInput").ap(),
            nc.dram_tensor("bkf", [128, G], F32, kind="ExternalInput").ap(),
            nc.dram_tensor("bo", [128, KT], F32, kind="ExternalInput").ap(),
            nc.dram_tensor("bvr", [1, FD], BF16, kind="ExternalInput").ap(),
            nc.dram_tensor("ident", [128, 128], BF16,
                           kind="ExternalInput").ap(),
            nc.dram_tensor("outT", [DIM, L], F32, kind="ExternalOutput").ap(),
        )
        with tile.TileContext(nc) as tc:
            _build_body(tc, io)
        nc.compile()
        _CACHED["nc"] = nc
    return _CACHED["nc"]


def _prep_maps(query, key, value, Wq, bq, Wk, bk, Wv, bv, Wo, bo):
    bf = ml_dtypes.bfloat16
    f32 = np.float32

    xT = {}
    for name, arr in (("q", query), ("k", key), ("v", value)):
        for b_idx in range(B):
            xT[(name, b_idx)] = np.ascontiguousarray(
                arr[b_idx].T.astype(bf)).reshape(KT, 128, L)

    halves = []
    for hh in range(2):
        cols = slice(hh * FD, (hh + 1) * FD)

        def foldw(W):
            # [1024, 512] local cols -> [128, G, KT, 128]
            wf = np.asarray(W, f32)[:, cols].astype(bf)
            return np.ascontiguousarray(
                wf.reshape(KT, 128, G, 128).transpose(1, 2, 0, 3))

        def foldb(b):
            bl = np.asarray(b, f32)[cols]
            return np.ascontiguousarray(bl.reshape(G, 128).T)

        halves.append({
            "wqf": foldw(Wq),
            "wkf": foldw(Wk),
            "wv": np.ascontiguousarray(
                np.asarray(Wv, f32)[:, cols].astype(bf)
                .reshape(KT, 128, FD).transpose(1, 0, 2)),
            "wo": np.ascontiguousarray(
                np.asarray(Wo, f32)[cols, :].astype(bf)
                .reshape(MT, 128, DIM).transpose(1, 0, 2)),
            "bqf": foldb(bq),
            "bkf": foldb(bk),
            "bvr": np.ascontiguousarray(
                np.asarray(bv, f32)[cols].astype(bf).reshape(1, FD)),
            "bo": np.ascontiguousarray(
                (np.asarray(bo, f32) if hh == 0 else
                 np.zeros(DIM, f32)).reshape(KT, 128).T),
        })
    ident = np.ascontiguousarray(np.eye(128, dtype=bf))
    in_maps = []
    for c in range(N_CORES):
        b_idx, hh = c // 2, c % 2
        in_maps.append(dict(
            halves[hh],
            ident=ident,
            xq=xT[("q", b_idx)], xk=xT[("k", b_idx)], xv=xT[("v", b_idx)],
        ))
    return in_maps


def kernel(query, key, value, Wq, bq, Wk, bk, Wv, bv, Wo, bo, **run_kwargs):
    query = np.asarray(query, np.float32)
    key = np.asarray(key, np.float32)
    value = np.asarray(value, np.float32)
    Wq, Wk, Wv, Wo = (np.asarray(w, np.float32) for w in (Wq, Wk, Wv, Wo))
    bq, bk, bv, bo = (np.asarray(b, np.float32) for b in (bq, bk, bv, bo))
    nc = _get_nc()
    in_maps = _prep_maps(query, key, value, Wq, bq, Wk, bk, Wv, bv, Wo, bo)
    res = bass_utils.run_bass_kernel_spmd(
        nc, in_maps, core_ids=list(range(N_CORES)), **run_kwargs)
    out = np.empty((B, L, DIM), np.float32)
    for b_idx in range(B):
        pa = res.results[2 * b_idx]["outT"]
        pb = res.results[2 * b_idx + 1]["outT"]
        out[b_idx] = (pa + pb).T
    _CACHED["last_results"] = res
    return out


# revision 45
# speedup vs baseline: 1.0159x; 1.0159x over previous
"""Multi-head attention layer on 8 TRN2 NeuronCores.

Reference computation (fp32):
    q = query @ Wq + bq; k = key @ Wk + bk; v = value @ Wv + bv
    scores = softmax(q @ k.T / sqrt(64)) per head
    out = (scores @ v) @ Wo + bo

Sharding: core c = 2*b + hh handles batch b and head-half hh (8 heads,
feature columns hh*512..(hh+1)*512). Each core computes its q/k/v
projections, attention for its 8 heads, and a partial output projection
against its 512-row slice of Wo; the host sums the two partials per batch.

Per-core design (cost-model-driven; TimelineSim 380.6us vs 436.6us for
the previous kernel):
  - bf16 everywhere (rel err 4.4e-3). qT/kT [128, 4, L] feature-major;
    head h lives at partitions (h%2)*64.. of feature tile h//2, so the
    K=64 score matmuls slice them directly.
  - exp on ScalarE in [128,1024] tiles (one per (head, lkt-pair,
    lq512-chunk)): 256 ACTs ~ 267us busy - the target engine to keep fed.
  - AV uses the o-layout: o[Lq, 64+1] accumulated per (head, lq128) as
    four 65-col strips in ONE PSUM bank. The first AV matmul opens the
    bank's psum accumulation group with start=True (hardware
    pending-zeroes the whole 2KB region, which is exactly what the other
    strips then accumulate onto); the last one closes it with stop=True.
    The ones column of v gives the softmax sums per partition, so
    normalization is one strided reciprocal [128,4] + 4 fused
    scale-copies (tensor_scalar mult) - no DRAM round trips.
  - oT for the output projection comes from PE transposes against a
    host-provided identity ([128,64] -> [64,128] bf16 into a rotating
    PSUM bank) + one DVE copy. (dma_start_transpose/XBAR produces wrong
    results on HW for <128-partition or non-contiguous destinations -
    it NaN'd here - so PE transpose it is.)
  - Head pairs alternate per kp step so each head's AV runs under the
    other head's exp; lq512-chunk outer/lkt inner keeps only 2
    accumulator banks live. k/v/q projections are emitted just-in-time
    along the attention frontier, extra work rides a filler queue pumped
    once per step.
PSUM: scores 2x[128,1024] (4 banks) + acc 2x[128,512] (2) +
proj/oproj/transpose 2x[128,512] (2) = 8 banks exactly.
"""

import numpy as np
import ml_dtypes

import concourse.bacc as bacc
import concourse.bass as bass
import concourse.mybir as mybir
import concourse.tile as tile
from concourse import bass_utils

B, L, DIM = 4, 2048, 1024
H, HD = 16, 64
N_CORES = 8
HL = 8             # local heads per core
FD = 512           # local feature columns
KT = DIM // 128    # 8 contraction k-tiles for projections
G = 4              # qF/kF free slots (head-dim sub-blocks)
MT = FD // 128     # 4 oT feature tiles
NLK = L // 128     # 16 Lk tiles
NC = L // 512      # 4 Lq column chunks
VSTR = 66          # per-head stride in v_sb (64 vals + ones col + pad)

BF16 = mybir.dt.bfloat16
FP8 = mybir.dt.float8e4
F32 = mybir.dt.float32
AF = mybir.ActivationFunctionType
DR = mybir.MatmulPerfMode.DoubleRow
ADD = mybir.AluOpType.add
SUB = mybir.AluOpType.subtract
MULT = mybir.AluOpType.mult


def _build_body(tc, io):
    nc = tc.nc
    (xq, xk, xv, wqf, wkf, wv, wo, bqf, bkf, bo, bvr, ident, outT) = io

    from contextlib import ExitStack
    with ExitStack() as ctx:
        const = ctx.enter_context(tc.tile_pool(name="const", bufs=1))
        wpool = ctx.enter_context(tc.tile_pool(name="wpool", bufs=1))
        xqpool = ctx.enter_context(tc.tile_pool(name="xqpool", bufs=2))
        xkpool = ctx.enter_context(tc.tile_pool(name="xkpool", bufs=4))
        xvpool = ctx.enter_context(tc.tile_pool(name="xvpool", bufs=2))
        qk_sb = ctx.enter_context(tc.tile_pool(name="qk_sb", bufs=1))
        e_pool = ctx.enter_context(tc.tile_pool(name="e_pool", bufs=6))
        osb_pool = ctx.enter_context(tc.tile_pool(name="osb", bufs=4))
        rec_pool = ctx.enter_context(tc.tile_pool(name="rec", bufs=4))
        stage = ctx.enter_context(tc.tile_pool(name="stage", bufs=4))
        spool = ctx.enter_context(
            tc.tile_pool(name="spool", bufs=2, space="PSUM"))
        apool = ctx.enter_context(
            tc.tile_pool(name="apool", bufs=2, space="PSUM"))
        ppool = ctx.enter_context(
            tc.tile_pool(name="ppool", bufs=2, space="PSUM"))

        # ---- constants (tiles now; DMAs deferred past the critical
        # q/k warmup loads -- they are only read a few us in) ----
        bq_sb = const.tile([128, G], F32)
        bk_sb = const.tile([128, G], F32)
        bo_sb = const.tile([128, KT], F32)
        bv_row = const.tile([1, FD], BF16)
        ones_col = const.tile([1, 128], BF16)
        ident_sb = const.tile([128, 128], BF16)

        def emit_consts():
            nc.sync.dma_start(out=bq_sb, in_=bqf)
            nc.sync.dma_start(out=bk_sb, in_=bkf)
            nc.sync.dma_start(out=bv_row, in_=bvr)
            nc.vector.memset(ones_col, 1.0)
            nc.sync.dma_start(out=ident_sb, in_=ident)
            nc.sync.dma_start(out=bo_sb, in_=bo)

        # ---- persistent activations ----
        qT = qk_sb.tile([128, G, L], BF16)
        kT = qk_sb.tile([128, G, L], BF16)
        v_sb = qk_sb.tile([128, NLK, HL * VSTR], BF16)
        oT_all = qk_sb.tile([128, MT, L], BF16)

        for h in range(HL):
            nc.vector.memset(v_sb[:, :, h * VSTR + 64:h * VSTR + 65], 1.0)

        # ---- weights: wq/wk [128, G, KT, 128] (per-g loads), wv/wo ----
        wq_sb = wpool.tile([128, G, KT, 128], BF16, tag="wq")
        wk_sb = wpool.tile([128, G, KT, 128], BF16, tag="wk")
        wv_sb = wpool.tile([128, KT, FD], BF16, tag="wv")
        wo_sb = wpool.tile([128, MT, DIM], BF16, tag="wo")
        wq_loaded = [False] * G
        wk_loaded = [False] * G
        wv_loaded = [False]
        wo_loaded = [False]

        def ensure_wq(g):
            if not wq_loaded[g]:
                nc.sync.dma_start(out=wq_sb[:, g], in_=wqf[:, g])
                wq_loaded[g] = True

        def ensure_wk(g):
            if not wk_loaded[g]:
                nc.sync.dma_start(out=wk_sb[:, g], in_=wkf[:, g])
                wk_loaded[g] = True

        def ensure_wv():
            if not wv_loaded[0]:
                nc.sync.dma_start(out=wv_sb, in_=wv)
                wv_loaded[0] = True

        def ensure_wo():
            if not wo_loaded[0]:
                nc.sync.dma_start(out=wo_sb, in_=wo)
                wo_loaded[0] = True

        # ---- x chunk loads (one DMA each: [128, KT, 512]) ----
        xq_ch, xk_ch, xv_ch = {}, {}, {}

        def load_chunk(pool, cache, dram, c, tag):
            if c not in cache:
                t = pool.tile([128, KT, 512], BF16, tag="x",
                              name=f"{tag}{c}")
                # one 3D DMA per chunk: dims iterated [p][kt][col]
                src = bass.AP(
                    tensor=dram.tensor, offset=dram.offset + c * 512,
                    ap=[[L, 128], [128 * L, KT], [1, 512]])
                nc.sync.dma_start(out=t, in_=src)
                cache[c] = t
            return cache[c]

        # ---- projection units (one PSUM bank each) ----
        qk_done = set()   # ("q"|"k", c_or_d, g)
        v_done = set()    # lkt

        def proj_unit(kind, c, g):
            """q or k projection for feature tile g, column chunk c."""
            if (kind, c, g) in qk_done:
                return
            qk_done.add((kind, c, g))
            if kind == "q":
                ensure_wq(g)
                x_t = load_chunk(xqpool, xq_ch, xq, c, "xq")
                w_t, b_t, dst = wq_sb, bq_sb, qT
            else:
                ensure_wk(g)
                x_t = load_chunk(xkpool, xk_ch, xk, c, "xk")
                w_t, b_t, dst = wk_sb, bk_sb, kT
            ps = ppool.tile([128, 512], F32, tag="p", name=f"{kind}p{c}{g}")
            for kt in range(KT):
                nc.tensor.matmul(ps, w_t[:, g, kt, :], x_t[:, kt, :],
                                 start=(kt == 0), stop=(kt == KT - 1))
            nc.vector.tensor_scalar(
                out=dst[:, g, c * 512:(c + 1) * 512], in0=ps,
                scalar1=b_t[:, g:g + 1], scalar2=None, op0=ADD)

        def vproj_unit(lkt):
            if lkt in v_done:
                return
            v_done.add(lkt)
            ensure_wv()
            x_t = load_chunk(xvpool, xv_ch, xv, lkt // 4, "xv")
            t = lkt % 4
            ps = ppool.tile([128, 512], F32, tag="p", name=f"vp{lkt}")
            for kt in range(KT):
                nc.tensor.matmul(ps, x_t[:, kt, t * 128:(t + 1) * 128],
                                 wv_sb[:, kt, :],
                                 start=(kt == 0), stop=False)
            nc.tensor.matmul(ps, ones_col, bv_row, start=False, stop=True)
            dst = v_sb[:, lkt, :].rearrange(
                "p (h d) -> p h d", d=VSTR)[:, :, 0:64]
            nc.vector.tensor_copy(
                out=dst, in_=ps.rearrange("p (h d) -> p h d", d=64))

        # ---- attention pieces ----
        def scores_exp(h, c, kp):
            """Scores (one K=64 bf16 matmul per lkt) + one [128,1024] exp."""
            mt, p0 = h // 2, (h % 2) * 64
            s_ps = spool.tile([128, 1024], F32, tag="s", name=f"s{h}{c}{kp}")
            for j in (0, 1):
                lkt = 2 * kp + j
                nc.tensor.matmul(
                    s_ps[:, j * 512:(j + 1) * 512],
                    kT[p0:p0 + 64, mt, lkt * 128:(lkt + 1) * 128],
                    qT[p0:p0 + 64, mt, c * 512:(c + 1) * 512],
                    start=True, stop=True)
            e_t = e_pool.tile([128, 2, 512], BF16, tag="e", name=f"e{h}{kp}")
            nc.scalar.activation(e_t.rearrange("p a b -> p (a b)"), s_ps,
                                 AF.Exp, scale=0.125)
            return e_t

        def av(h, acc, e_t, kp):
            # the (kp0, j0, sub0) matmul opens the bank's psum group
            # (pending-zeroing the whole 2KB region, which is what the
            # other strips then accumulate onto); the last one closes it
            for j in (0, 1):
                lkt = 2 * kp + j
                va = v_sb[:, lkt, h * VSTR:h * VSTR + 65]
                for sub in range(4):
                    first = kp == 0 and j == 0 and sub == 0
                    last = kp == 7 and j == 1 and sub == 3
                    nc.tensor.matmul(
                        acc[:, sub * 128:sub * 128 + 65],
                        e_t[:, j, sub * 128:(sub + 1) * 128], va,
                        start=first, stop=last,
                        skip_group_check=not (first or last))

        def norm_transpose(h, c, acc):
            """1/sums, scale, and XBAR-transpose into oT_all."""
            rec4 = rec_pool.tile([128, G, 1], F32, tag="r", name=f"r{h}{c}")
            sums = acc.rearrange("p (s x) -> p s x", x=128)[:, :, 64:65]
            nc.vector.reciprocal(out=rec4, in_=sums)
            o_sb = osb_pool.tile([128, 4, 64], BF16, tag="o",
                                 name=f"o{h}{c}")
            for sub in range(4):
                nc.vector.tensor_scalar(
                    out=o_sb[:, sub, :],
                    in0=acc[:, sub * 128:sub * 128 + 64],
                    scalar1=rec4[:, sub, :], scalar2=None, op0=MULT)
            hp = (h % 2) * 64
            # PE transpose via identity: o_sb [128,64] -> [64,128] in PSUM
            tp = ppool.tile([128, 1024], BF16, tag="p", name=f"tp{h}{c}")
            for sub in range(4):
                nc.tensor.transpose(
                    tp[hp:hp + 64, sub * 128:(sub + 1) * 128],
                    o_sb[:, sub, :], ident_sb)
            nc.vector.tensor_copy(
                out=oT_all[hp:hp + 64, h // 2, c * 512:(c + 1) * 512],
                in_=tp[hp:hp + 64, 0:512])

        def oproj_unit(c, mt):
            ps = ppool.tile([128, 512], F32, tag="p", name=f"op{c}{mt}")
            for kt in range(MT):
                nc.tensor.matmul(ps, wo_sb[:, kt, mt * 128:(mt + 1) * 128],
                                 oT_all[:, kt, c * 512:(c + 1) * 512],
                                 start=(kt == 0), stop=(kt == MT - 1))
            st = stage.tile([128, 512], F32, tag="st", name=f"st{c}{mt}")
            nc.vector.tensor_scalar(
                out=st, in0=ps, scalar1=bo_sb[:, mt:mt + 1], scalar2=None,
                op0=ADD)
            nc.sync.dma_start(
                out=outT[mt * 128:(mt + 1) * 128, c * 512:(c + 1) * 512],
                in_=st)

        # ---- orchestration ----
        fillers = []

        def pump(n=1):
            for _ in range(min(n, len(fillers))):
                fillers.pop(0)()

        # c=0 warmup front: just enough for the first ACT + first AV
        proj_unit("q", 0, 0)
        proj_unit("k", 0, 0)
        emit_consts()
        vproj_unit(0)
        vproj_unit(1)
        vproj_unit(2)
        vproj_unit(3)

        for c in range(NC):
            if c == 0:
                # remaining q slots of chunk 0 (before any xq recycling),
                # then wo for the first oproj units
                for g in range(1, G):
                    fillers.append(lambda g=g: proj_unit("q", 0, g))
            else:
                # any stragglers (normally already pumped as fillers)
                for g in range(G):
                    proj_unit("q", c, g)
                for mt in range(KT):
                    fillers.append(lambda c=c, mt=mt: oproj_unit(c - 1, mt))
            if c + 1 < NC:
                for g in range(G):
                    fillers.append(
                        lambda c=c, g=g: proj_unit("q", c + 1, g))
            if c == 0:
                fillers.append(ensure_wo)

            for pair in range(HL // 2):
                hA, hB = 2 * pair, 2 * pair + 1
                accs = {}
                for h in (hA, hB):
                    # zeroing comes from the first AV matmul's start=True
                    # (pending-zeroes the whole bank region)
                    accs[h] = apool.tile([128, 512], F32, tag="a",
                                         name=f"acc{h}{c}")
                pend = []
                for kp in range(HL):
                    if c == 0:
                        # JIT: kproj for this pair's scores, v for the AVs
                        proj_unit("k", kp // 2, pair)
                        vproj_unit(2 * kp)
                        vproj_unit(2 * kp + 1)
                    for h in (hA, hB):
                        e_t = scores_exp(h, c, kp)
                        pend.append((h, e_t, kp))
                    # AV one step behind: both heads of the previous kp
                    while len(pend) > 4:
                        h, e_t, kpp = pend.pop(0)
                        av(h, accs[h], e_t, kpp)
                    pump(1)
                for h, e_t, kpp in pend:
                    av(h, accs[h], e_t, kpp)
                norm_transpose(hA, c, accs[hA])
                norm_transpose(hB, c, accs[hB])

        for mt in range(KT):
            oproj_unit(NC - 1, mt)
        while fillers:
            pump(1)


_CACHED = {}


def _get_nc():
    if "nc" not in _CACHED:
        nc = bacc.Bacc("TRN2", target_bir_lowering=False, debug=False)
        io = (
            nc.dram_tensor("xq", [KT, 128, L], BF16, kind="ExternalInput").ap(),
            nc.dram_tensor("xk", [KT, 128, L], BF16, kind="ExternalInput").ap(),
            nc.dram_tensor("xv", [KT, 128, L], BF16, kind="ExternalInput").ap(),
            nc.dram_tensor("wqf", [128, G, KT, 128], BF16,
                           kind="ExternalInput").ap(),
            nc.dram_tensor("wkf", [128, G, KT, 128], BF16,
                           kind="ExternalInput").ap(),
            nc.dram_tensor("wv", [128, KT, FD], BF16,
                           kind="ExternalInput").ap(),
            nc.dram_tensor("wo", [128, MT, DIM], BF16,
                           kind="ExternalInput").ap(),
            nc.dram_tensor("bqf", [128, G], F32, kind="ExternalInput").ap(),
            nc.dram_tensor("bkf", [128, G], F32, kind="ExternalInput").ap(),
            nc.dram_tensor("bo", [128, KT], F32, kind="ExternalInput").ap(),
            nc.dram_tensor("bvr", [1, FD], BF16, kind="ExternalInput").ap(),
            nc.dram_tensor("ident", [128, 128], BF16,
                           kind="ExternalInput").ap(),
            nc.dram_tensor("outT", [DIM, L], F32, kind="ExternalOutput").ap(),
        )
        with tile.TileContext(nc) as tc:
            _build_body(tc, io)
        nc.compile()
        _CACHED["nc"] = nc
    return _CACHED["nc"]


def _prep_maps(query, key, value, Wq, bq, Wk, bk, Wv, bv, Wo, bo):
    bf = ml_dtypes.bfloat16
    f32 = np.float32

    xT = {}
    for name, arr in (("q", query), ("k", key), ("v", value)):
        for b_idx in range(B):
            xT[(name, b_idx)] = np.ascontiguousarray(
                arr[b_idx].T.astype(bf)).reshape(KT, 128, L)

    halves = []
    for hh in range(2):
        cols = slice(hh * FD, (hh + 1) * FD)

        def foldw(W):
            # [1024, 512] local cols -> [128, G, KT, 128]
            wf = np.asarray(W, f32)[:, cols].astype(bf)
            return np.ascontiguousarray(
                wf.reshape(KT, 128, G, 128).transpose(1, 2, 0, 3))

        def foldb(b):
            bl = np.asarray(b, f32)[cols]
            return np.ascontiguousarray(bl.reshape(G, 128).T)

        halves.append({
            "wqf": foldw(Wq),
            "wkf": foldw(Wk),
            "wv": np.ascontiguousarray(
                np.asarray(Wv, f32)[:, cols].astype(bf)
                .reshape(KT, 128, FD).transpose(1, 0, 2)),
            "wo": np.ascontiguousarray(
                np.asarray(Wo, f32)[cols, :].astype(bf)
                .reshape(MT, 128, DIM).transpose(1, 0, 2)),
            "bqf": foldb(bq),
            "bkf": foldb(bk),
            "bvr": np.ascontiguousarray(
                np.asarray(bv, f32)[cols].astype(bf).reshape(1, FD)),
            "bo": np.ascontiguousarray(
                (np.asarray(bo, f32) if hh == 0 else
                 np.zeros(DIM, f32)).reshape(KT, 128).T),
        })
    ident = np.ascontiguousarray(np.eye(128, dtype=bf))
    in_maps = []
    for c in range(N_CORES):
        b_idx, hh = c // 2, c % 2
        in_maps.append(dict(
            halves[hh],
            ident=ident,
            xq=xT[("q", b_idx)], xk=xT[("k", b_idx)], xv=xT[("v", b_idx)],
        ))
    return in_maps


def kernel(query, key, value, Wq, bq, Wk, bk, Wv, bv, Wo, bo, **run_kwargs):
    query = np.asarray(query, np.float32)
    key = np.asarray(key, np.float32)
    value = np.asarray(value, np.float32)
    Wq, Wk, Wv, Wo = (np.asarray(w, np.float32) for w in (Wq, Wk, Wv, Wo))
    bq, bk, bv, bo = (np.asarray(b, np.float32) for b in (bq, bk, bv, bo))
    nc = _get_nc()
    in_maps = _prep_maps(query, key, value, Wq, bq, Wk, bk, Wv, bv, Wo, bo)
    res = bass_utils.run_bass_kernel_spmd(
        nc, in_maps, core_ids=list(range(N_CORES)), **run_kwargs)
    out = np.empty((B, L, DIM), np.float32)
    for b_idx in range(B):
        pa = res.results[2 * b_idx]["outT"]
        pb = res.results[2 * b_idx + 1]["outT"]
        out[b_idx] = (pa + pb).T
    _CACHED["last_results"] = res
    return out


# revision 47
# speedup vs baseline: 1.0192x; 1.0033x over previous
"""Multi-head attention layer on 8 TRN2 NeuronCores.

Reference computation (fp32):
    q = query @ Wq + bq; k = key @ Wk + bk; v = value @ Wv + bv
    scores = softmax(q @ k.T / sqrt(64)) per head
    out = (scores @ v) @ Wo + bo

Sharding: core c = 2*b + hh handles batch b and head-half hh (8 heads,
feature columns hh*512..(hh+1)*512). Each core computes its q/k/v
projections, attention for its 8 heads, and a partial output projection
against its 512-row slice of Wo; the host sums the two partials per batch.

Per-core design (cost-model-driven; TimelineSim 380.6us vs 436.6us for
the previous kernel):
  - bf16 everywhere (rel err 4.4e-3). qT/kT [128, 4, L] feature-major;
    head h lives at partitions (h%2)*64.. of feature tile h//2, so the
    K=64 score matmuls slice them directly.
  - exp on ScalarE in [128,1024] tiles (one per (head, lkt-pair,
    lq512-chunk)): 256 ACTs ~ 267us busy - the target engine to keep fed.
  - AV uses the o-layout: o[Lq, 64+1] accumulated per (head, lq128) as
    four 65-col strips in ONE PSUM bank. The first AV matmul opens the
    bank's psum accumulation group with start=True (hardware
    pending-zeroes the whole 2KB region, which is exactly what the other
    strips then accumulate onto); the last one closes it with stop=True.
    The ones column of v gives the softmax sums per partition, so
    normalization is one strided reciprocal [128,4] + 4 fused
    scale-copies (tensor_scalar mult) - no DRAM round trips.
  - oT for the output projection comes from PE transposes against a
    host-provided identity ([128,64] -> [64,128] bf16 into a rotating
    PSUM bank) + one DVE copy. (dma_start_transpose/XBAR produces wrong
    results on HW for <128-partition or non-contiguous destinations -
    it NaN'd here - so PE transpose it is.)
  - Head pairs alternate per kp step so each head's AV runs under the
    other head's exp; lq512-chunk outer/lkt inner keeps only 2
    accumulator banks live. k/v/q projections are emitted just-in-time
    along the attention frontier, extra work rides a filler queue pumped
    once per step.
PSUM: scores 2x[128,1024] (4 banks) + acc 2x[128,512] (2) +
proj/oproj/transpose 2x[128,512] (2) = 8 banks exactly.
"""

import numpy as np
import ml_dtypes

import concourse.bacc as bacc
import concourse.bass as bass
import concourse.mybir as mybir
import concourse.tile as tile
from concourse import bass_utils

B, L, DIM = 4, 2048, 1024
H, HD = 16, 64
N_CORES = 8
HL = 8             # local heads per core
FD = 512           # local feature columns
KT = DIM // 128    # 8 contraction k-tiles for projections
G = 4              # qF/kF free slots (head-dim sub-blocks)
MT = FD // 128     # 4 oT feature tiles
NLK = L // 128     # 16 Lk tiles
NC = L // 512      # 4 Lq column chunks
VSTR = 66          # per-head stride in v_sb (64 vals + ones col + pad)

BF16 = mybir.dt.bfloat16
FP8 = mybir.dt.float8e4
F32 = mybir.dt.float32
AF = mybir.ActivationFunctionType
DR = mybir.MatmulPerfMode.DoubleRow
ADD = mybir.AluOpType.add
SUB = mybir.AluOpType.subtract
MULT = mybir.AluOpType.mult


def _build_body(tc, io):
    nc = tc.nc
    (xq, xk, xv, wqf, wkf, wv, wo, bqf, bkf, bo, bvr, ident, outT) = io

    from contextlib import ExitStack
    with ExitStack() as ctx:
        const = ctx.enter_context(tc.tile_pool(name="const", bufs=1))
        wpool = ctx.enter_context(tc.tile_pool(name="wpool", bufs=1))
        xqpool = ctx.enter_context(tc.tile_pool(name="xqpool", bufs=2))
        xkpool = ctx.enter_context(tc.tile_pool(name="xkpool", bufs=4))
        xvpool = ctx.enter_context(tc.tile_pool(name="xvpool", bufs=2))
        qk_sb = ctx.enter_context(tc.tile_pool(name="qk_sb", bufs=1))
        e_pool = ctx.enter_context(tc.tile_pool(name="e_pool", bufs=6))
        osb_pool = ctx.enter_context(tc.tile_pool(name="osb", bufs=4))
        rec_pool = ctx.enter_context(tc.tile_pool(name="rec", bufs=4))
        stage = ctx.enter_context(tc.tile_pool(name="stage", bufs=4))
        spool = ctx.enter_context(
            tc.tile_pool(name="spool", bufs=2, space="PSUM"))
        apool = ctx.enter_context(
            tc.tile_pool(name="apool", bufs=2, space="PSUM"))
        ppool = ctx.enter_context(
            tc.tile_pool(name="ppool", bufs=2, space="PSUM"))

        # ---- constants (tiles now; DMAs deferred past the critical
        # q/k warmup loads -- they are only read a few us in) ----
        bq_sb = const.tile([128, G], F32)
        bk_sb = const.tile([128, G], F32)
        bo_sb = const.tile([128, KT], F32)
        bv_row = const.tile([1, FD], BF16)
        ones_col = const.tile([1, 128], BF16)
        ident_sb = const.tile([128, 128], BF16)

        def emit_biases():
            # must precede the first proj_unit emission: the tile
            # framework only orders writer-before-reader
            nc.sync.dma_start(out=bq_sb, in_=bqf)
            nc.sync.dma_start(out=bk_sb, in_=bkf)

        def emit_consts():
            nc.sync.dma_start(out=bv_row, in_=bvr)
            nc.vector.memset(ones_col, 1.0)
            nc.sync.dma_start(out=ident_sb, in_=ident)
            nc.sync.dma_start(out=bo_sb, in_=bo)

        # ---- persistent activations ----
        qT = qk_sb.tile([128, G, L], BF16)
        kT = qk_sb.tile([128, G, L], BF16)
        v_sb = qk_sb.tile([128, NLK, HL * VSTR], BF16)
        oT_all = qk_sb.tile([128, MT, L], BF16)

        for h in range(HL):
            nc.vector.memset(v_sb[:, :, h * VSTR + 64:h * VSTR + 65], 1.0)

        # ---- weights: wq/wk [128, G, KT, 128] (per-g loads), wv/wo ----
        wq_sb = wpool.tile([128, G, KT, 128], BF16, tag="wq")
        wk_sb = wpool.tile([128, G, KT, 128], BF16, tag="wk")
        wv_sb = wpool.tile([128, KT, FD], BF16, tag="wv")
        wo_sb = wpool.tile([128, MT, DIM], BF16, tag="wo")
        wq_loaded = [False] * G
        wk_loaded = [False] * G
        wv_loaded = [False]
        wo_loaded = [False]

        def ensure_wq(g):
            if not wq_loaded[g]:
                nc.sync.dma_start(out=wq_sb[:, g], in_=wqf[:, g])
                wq_loaded[g] = True

        def ensure_wk(g):
            if not wk_loaded[g]:
                nc.sync.dma_start(out=wk_sb[:, g], in_=wkf[:, g])
                wk_loaded[g] = True

        def ensure_wv():
            if not wv_loaded[0]:
                nc.sync.dma_start(out=wv_sb, in_=wv)
                wv_loaded[0] = True

        def ensure_wo():
            if not wo_loaded[0]:
                nc.sync.dma_start(out=wo_sb, in_=wo)
                wo_loaded[0] = True

        # ---- x chunk loads (one DMA each: [128, KT, 512]) ----
        xq_ch, xk_ch, xv_ch = {}, {}, {}

        def load_chunk(pool, cache, dram, c, tag):
            if c not in cache:
                t = pool.tile([128, KT, 512], BF16, tag="x",
                              name=f"{tag}{c}")
                # one 3D DMA per chunk: dims iterated [p][kt][col]
                src = bass.AP(
                    tensor=dram.tensor, offset=dram.offset + c * 512,
                    ap=[[L, 128], [128 * L, KT], [1, 512]])
                nc.sync.dma_start(out=t, in_=src)
                cache[c] = t
            return cache[c]

        # ---- projection units (one PSUM bank each) ----
        qk_done = set()   # ("q"|"k", c_or_d, g)
        v_done = set()    # lkt

        def proj_unit(kind, c, g):
            """q or k projection for feature tile g, column chunk c."""
            if (kind, c, g) in qk_done:
                return
            qk_done.add((kind, c, g))
            if kind == "q":
                ensure_wq(g)
                x_t = load_chunk(xqpool, xq_ch, xq, c, "xq")
                w_t, b_t, dst = wq_sb, bq_sb, qT
            else:
                ensure_wk(g)
                x_t = load_chunk(xkpool, xk_ch, xk, c, "xk")
                w_t, b_t, dst = wk_sb, bk_sb, kT
            ps = ppool.tile([128, 512], F32, tag="p", name=f"{kind}p{c}{g}")
            for kt in range(KT):
                nc.tensor.matmul(ps, w_t[:, g, kt, :], x_t[:, kt, :],
                                 start=(kt == 0), stop=(kt == KT - 1))
            nc.vector.tensor_scalar(
                out=dst[:, g, c * 512:(c + 1) * 512], in0=ps,
                scalar1=b_t[:, g:g + 1], scalar2=None, op0=ADD)

        def vproj_unit(lkt):
            if lkt in v_done:
                return
            v_done.add(lkt)
            ensure_wv()
            x_t = load_chunk(xvpool, xv_ch, xv, lkt // 4, "xv")
            t = lkt % 4
            ps = ppool.tile([128, 512], F32, tag="p", name=f"vp{lkt}")
            for kt in range(KT):
                nc.tensor.matmul(ps, x_t[:, kt, t * 128:(t + 1) * 128],
                                 wv_sb[:, kt, :],
                                 start=(kt == 0), stop=False)
            nc.tensor.matmul(ps, ones_col, bv_row, start=False, stop=True)
            dst = v_sb[:, lkt, :].rearrange(
                "p (h d) -> p h d", d=VSTR)[:, :, 0:64]
            nc.vector.tensor_copy(
                out=dst, in_=ps.rearrange("p (h d) -> p h d", d=64))

        # ---- attention pieces ----
        def scores_exp(h, c, kp):
            """Scores (one K=64 bf16 matmul per lkt) + one [128,1024] exp."""
            mt, p0 = h // 2, (h % 2) * 64
            s_ps = spool.tile([128, 1024], F32, tag="s", name=f"s{h}{c}{kp}")
            for j in (0, 1):
                lkt = 2 * kp + j
                nc.tensor.matmul(
                    s_ps[:, j * 512:(j + 1) * 512],
                    kT[p0:p0 + 64, mt, lkt * 128:(lkt + 1) * 128],
                    qT[p0:p0 + 64, mt, c * 512:(c + 1) * 512],
                    start=True, stop=True)
            e_t = e_pool.tile([128, 2, 512], BF16, tag="e", name=f"e{h}{kp}")
            nc.scalar.activation(e_t.rearrange("p a b -> p (a b)"), s_ps,
                                 AF.Exp, scale=0.125)
            return e_t

        def av(h, acc, e_t, kp):
            # the (kp0, j0, sub0) matmul opens the bank's psum group
            # (pending-zeroing the whole 2KB region, which is what the
            # other strips then accumulate onto); the last one closes it
            for j in (0, 1):
                lkt = 2 * kp + j
                va = v_sb[:, lkt, h * VSTR:h * VSTR + 65]
                for sub in range(4):
                    first = kp == 0 and j == 0 and sub == 0
                    last = kp == 7 and j == 1 and sub == 3
                    nc.tensor.matmul(
                        acc[:, sub * 128:sub * 128 + 65],
                        e_t[:, j, sub * 128:(sub + 1) * 128], va,
                        start=first, stop=last,
                        skip_group_check=not (first or last))

        def norm_transpose(h, c, acc):
            """1/sums, scale, and XBAR-transpose into oT_all."""
            rec4 = rec_pool.tile([128, G, 1], F32, tag="r", name=f"r{h}{c}")
            sums = acc.rearrange("p (s x) -> p s x", x=128)[:, :, 64:65]
            nc.vector.reciprocal(out=rec4, in_=sums)
            o_sb = osb_pool.tile([128, 4, 64], BF16, tag="o",
                                 name=f"o{h}{c}")
            for sub in range(4):
                nc.vector.tensor_scalar(
                    out=o_sb[:, sub, :],
                    in0=acc[:, sub * 128:sub * 128 + 64],
                    scalar1=rec4[:, sub, :], scalar2=None, op0=MULT)
            hp = (h % 2) * 64
            # PE transpose via identity: o_sb [128,64] -> [64,128] in PSUM
            tp = ppool.tile([128, 1024], BF16, tag="p", name=f"tp{h}{c}")
            for sub in range(4):
                nc.tensor.transpose(
                    tp[hp:hp + 64, sub * 128:(sub + 1) * 128],
                    o_sb[:, sub, :], ident_sb)
            nc.vector.tensor_copy(
                out=oT_all[hp:hp + 64, h // 2, c * 512:(c + 1) * 512],
                in_=tp[hp:hp + 64, 0:512])

        def oproj_unit(c, mt):
            ps = ppool.tile([128, 512], F32, tag="p", name=f"op{c}{mt}")
            for kt in range(MT):
                nc.tensor.matmul(ps, wo_sb[:, kt, mt * 128:(mt + 1) * 128],
                                 oT_all[:, kt, c * 512:(c + 1) * 512],
                                 start=(kt == 0), stop=(kt == MT - 1))
            st = stage.tile([128, 512], F32, tag="st", name=f"st{c}{mt}")
            nc.vector.tensor_scalar(
                out=st, in0=ps, scalar1=bo_sb[:, mt:mt + 1], scalar2=None,
                op0=ADD)
            nc.sync.dma_start(
                out=outT[mt * 128:(mt + 1) * 128, c * 512:(c + 1) * 512],
                in_=st)

        # ---- orchestration ----
        fillers = []

        def pump(n=1):
            for _ in range(min(n, len(fillers))):
                fillers.pop(0)()

        # c=0 warmup front: just enough for the first ACT + first AV
        emit_biases()
        proj_unit("q", 0, 0)
        proj_unit("k", 0, 0)
        emit_consts()
        vproj_unit(0)
        vproj_unit(1)
        vproj_unit(2)
        vproj_unit(3)

        for c in range(NC):
            if c == 0:
                # remaining q slots of chunk 0 (before any xq recycling),
                # then wo for the first oproj units
                for g in range(1, G):
                    fillers.append(lambda g=g: proj_unit("q", 0, g))
            else:
                # any stragglers (normally already pumped as fillers)
                for g in range(G):
                    proj_unit("q", c, g)
                for mt in range(KT):
                    fillers.append(lambda c=c, mt=mt: oproj_unit(c - 1, mt))
            if c + 1 < NC:
                for g in range(G):
                    fillers.append(
                        lambda c=c, g=g: proj_unit("q", c + 1, g))
            if c == 0:
                fillers.append(ensure_wo)

            for pair in range(HL // 2):
                hA, hB = 2 * pair, 2 * pair + 1
                accs = {}
                for h in (hA, hB):
                    # zeroing comes from the first AV matmul's start=True
                    # (pending-zeroes the whole bank region)
                    accs[h] = apool.tile([128, 512], F32, tag="a",
                                         name=f"acc{h}{c}")
                pend = []
                for kp in range(HL):
                    if c == 0:
                        # JIT: kproj for this pair's scores, v for the AVs
                        proj_unit("k", kp // 2, pair)
                        vproj_unit(2 * kp)
                        vproj_unit(2 * kp + 1)
                    for h in (hA, hB):
                        e_t = scores_exp(h, c, kp)
                        pend.append((h, e_t, kp))
                    # AV one step behind: both heads of the previous kp
                    while len(pend) > 4:
                        h, e_t, kpp = pend.pop(0)
                        av(h, accs[h], e_t, kpp)
                    pump(1)
                for h, e_t, kpp in pend:
                    av(h, accs[h], e_t, kpp)
                norm_transpose(hA, c, accs[hA])
                norm_transpose(hB, c, accs[hB])

        for mt in range(KT):
            oproj_unit(NC - 1, mt)
        while fillers:
            pump(1)


_CACHED = {}


def _get_nc():
    if "nc" not in _CACHED:
        nc = bacc.Bacc("TRN2", target_bir_lowering=False, debug=False)
        io = (
            nc.dram_tensor("xq", [KT, 128, L], BF16, kind="ExternalInput").ap(),
            nc.dram_tensor("xk", [KT, 128, L], BF16, kind="ExternalInput").ap(),
            nc.dram_tensor("xv", [KT, 128, L], BF16, kind="ExternalInput").ap(),
            nc.dram_tensor("wqf", [128, G, KT, 128], BF16,
                           kind="ExternalInput").ap(),
            nc.dram_tensor("wkf", [128, G, KT, 128], BF16,
                           kind="ExternalInput").ap(),
            nc.dram_tensor("wv", [128, KT, FD], BF16,
                           kind="ExternalInput").ap(),
            nc.dram_tensor("wo", [128, MT, DIM], BF16,
                           kind="ExternalInput").ap(),
            nc.dram_tensor("bqf", [128, G], F32, kind="ExternalInput").ap(),
            nc.dram_tensor("bkf", [128, G], F32, kind="ExternalInput").ap(),
            nc.dram_tensor("bo", [128, KT], F32, kind="ExternalInput").ap(),
            nc.dram_tensor("bvr", [1, FD], BF16, kind="ExternalInput").ap(),
            nc.dram_tensor("ident", [128, 128], BF16,
                           kind="ExternalInput").ap(),
            nc.dram_tensor("outT", [DIM, L], F32, kind="ExternalOutput").ap(),
        )
        with tile.TileContext(nc) as tc:
            _build_body(tc, io)
        nc.compile()
        _CACHED["nc"] = nc
    return _CACHED["nc"]


def _prep_maps(query, key, value, Wq, bq, Wk, bk, Wv, bv, Wo, bo):
    bf = ml_dtypes.bfloat16
    f32 = np.float32

    xT = {}
    for name, arr in (("q", query), ("k", key), ("v", value)):
        for b_idx in range(B):
            xT[(name, b_idx)] = np.ascontiguousarray(
                arr[b_idx].T.astype(bf)).reshape(KT, 128, L)

    halves = []
    for hh in range(2):
        cols = slice(hh * FD, (hh + 1) * FD)

        def foldw(W):
            # [1024, 512] local cols -> [128, G, KT, 128]
            wf = np.asarray(W, f32)[:, cols].astype(bf)
            return np.ascontiguousarray(
                wf.reshape(KT, 128, G, 128).transpose(1, 2, 0, 3))

        def foldb(b):
            bl = np.asarray(b, f32)[cols]
            return np.ascontiguousarray(bl.reshape(G, 128).T)

        halves.append({
            "wqf": foldw(Wq),
            "wkf": foldw(Wk),
            "wv": np.ascontiguousarray(
                np.asarray(Wv, f32)[:, cols].astype(bf)
                .reshape(KT, 128, FD).transpose(1, 0, 2)),
            "wo": np.ascontiguousarray(
                np.asarray(Wo, f32)[cols, :].astype(bf)
                .reshape(MT, 128, DIM).transpose(1, 0, 2)),
            "bqf": foldb(bq),
            "bkf": foldb(bk),
            "bvr": np.ascontiguousarray(
                np.asarray(bv, f32)[cols].astype(bf).reshape(1, FD)),
            "bo": np.ascontiguousarray(
                (np.asarray(bo, f32) if hh == 0 else
                 np.zeros(DIM, f32)).reshape(KT, 128).T),
        })
    ident = np.ascontiguousarray(np.eye(128, dtype=bf))
    in_maps = []
    for c in range(N_CORES):
        b_idx, hh = c // 2, c % 2
        in_maps.append(dict(
            halves[hh],
            ident=ident,
            xq=xT[("q", b_idx)], xk=xT[("k", b_idx)], xv=xT[("v", b_idx)],
        ))
    return in_maps


def kernel(query, key, value, Wq, bq, Wk, bk, Wv, bv, Wo, bo, **run_kwargs):
    query = np.asarray(query, np.float32)
    key = np.asarray(key, np.float32)
    value = np.asarray(value, np.float32)
    Wq, Wk, Wv, Wo = (np.asarray(w, np.float32) for w in (Wq, Wk, Wv, Wo))
    bq, bk, bv, bo = (np.asarray(b, np.float32) for b in (bq, bk, bv, bo))
    nc = _get_nc()
    in_maps = _prep_maps(query, key, value, Wq, bq, Wk, bk, Wv, bv, Wo, bo)
    res = bass_utils.run_bass_kernel_spmd(
        nc, in_maps, core_ids=list(range(N_CORES)), **run_kwargs)
    out = np.empty((B, L, DIM), np.float32)
    for b_idx in range(B):
        pa = res.results[2 * b_idx]["outT"]
        pb = res.results[2 * b_idx + 1]["outT"]
        out[b_idx] = (pa + pb).T
    _CACHED["last_results"] = res
    return out
